# revision 5
# baseline (speedup 1.0000x reference)
"""Contrastive loss (batch-hard triplet, within batch) on 8 Trainium2 cores.

Math (matches the jax reference):
    xn = x / ||x||_2 (rows)                      [B, C] = [4096, 1024]
    g[i,j] = xn_i . xn_j
    d[i,j] = max(2 - 2 g, 0)   (since ||xn||=1)
    pos_i  = sum_{j: same label, j != i} d[i,j]
    neg_i  = min_{j: diff label} d[i,j]
    loss   = mean(relu(pos_i + 0.5 - neg_i))

Sharding: rows (anchors) split 512/core. Every core builds the full
normalized-transposed xn^T (bf16) and computes its [512, 4096] tile of m.

Label-mask fusion: feature dim is augmented with 64 one-hot columns scaled
+8 on the anchor side and -8 on the rhs side, so the PE produces
    m[i,j] = g[i,j] - 64 * same[i,j]
in the same accumulation group.  Then
    pos_i     = 2 * sum_j relu(-m - 63)          (exact: relu kills diff-label)
    hardest_i = 2 * relu(1 - max_j m)            (max ignores same-label, m<=-63)
    loss_i    = relu(2*(pos_half - hardest_half) + 0.5)
Per-core output is sum(loss_i)/4096; the host adds the 8 partial scalars.
"""

import sys

if "/opt/trn_rl_repo" not in sys.path:
    sys.path.insert(0, "/opt/trn_rl_repo")

from contextlib import ExitStack

import ml_dtypes
import numpy as np

import concourse.bass as bass
import concourse.tile as tile
from concourse import bacc, mybir
from concourse.bass_utils import run_bass_kernel_spmd

B = 4096          # batch rows
C = 1024          # features
NCORES = 8
BA = B // NCORES  # anchors per core = 512
P = 128
KC = C // P       # 8 feature chunks of 128
NT = B // P       # 32 row tiles of full x
NTA = BA // P     # 4 row tiles of anchors
NM = BA // P      # 4 anchor blocks (M=128 each)
NB = 512          # j-block width (moving free dim)
NJ = B // NB      # 8 j blocks
NLAB = 64
ALPHA = 8.0       # onehot scale; ALPHA^2 = 64 = label-offset

F32 = mybir.dt.float32
BF16 = mybir.dt.bfloat16
AF = mybir.ActivationFunctionType
AX = mybir.AxisListType


def build_kernel():
    nc = bacc.Bacc("TRN2", target_bir_lowering=False, debug=False,
                   num_devices=NCORES)
    x_d = nc.dram_tensor("x", (B, C), F32, kind="ExternalInput").ap()
    xa_d = nc.dram_tensor("xa", (BA, C), F32, kind="ExternalInput").ap()
    ohp_d = nc.dram_tensor("ohp", (NLAB, BA), BF16, kind="ExternalInput").ap()
    ohn_d = nc.dram_tensor("ohn", (NLAB, B), BF16, kind="ExternalInput").ap()
    out_d = nc.dram_tensor("out", (1, 1), F32, kind="ExternalOutput").ap()

    with tile.TileContext(nc) as tc, ExitStack() as ctx:
        big = ctx.enter_context(tc.tile_pool(name="big", bufs=1))
        xload = ctx.enter_context(tc.tile_pool(name="xload", bufs=4))
        xnp_ = ctx.enter_context(tc.tile_pool(name="xnp", bufs=4))
        stats = ctx.enter_context(tc.tile_pool(name="stats", bufs=8))
        scratch = ctx.enter_context(tc.tile_pool(name="scratch", bufs=2))
        psum = ctx.enter_context(tc.tile_pool(name="psum", bufs=7, space="PSUM"))
        psum1 = ctx.enter_context(tc.tile_pool(name="psum1", bufs=1, space="PSUM"))
        small = ctx.enter_context(tc.tile_pool(name="small", bufs=1))

        # Persistent transposed tensors.
        # xnt[p, t, c, j'] = xn[t*128 + j', c*128 + p]  (bf16)
        xnt = big.tile([P, NT, KC, P], BF16)
        xat = big.tile([P, NTA, KC, P], BF16)
        ohp = big.tile([NLAB, BA], BF16)
        ohn = big.tile([NLAB, B], BF16)
        pos_all = big.tile([P, NM * NJ], F32)
        max_all = big.tile([P, NM * NJ], F32)
        ones = big.tile([P, 1], F32)
        bneg63 = big.tile([P, 1], F32)
        bhalf = big.tile([P, 1], F32)

        nc.sync.dma_start(ohp[:], ohp_d)
        nc.sync.dma_start(ohn[:], ohn_d)
        nc.vector.memset(ones[:], 1.0)
        nc.vector.memset(bneg63[:], -63.0)
        nc.vector.memset(bhalf[:], 0.5)

        # ---- prep: load rows, normalize, bf16, xbar-transpose ----
        for t in range(NT + NTA):
            if t < NT:
                src = x_d[t * P:(t + 1) * P, :]
                dst = xnt[:, t, :, :]
            else:
                src = xa_d[(t - NT) * P:(t - NT + 1) * P, :]
                dst = xat[:, t - NT, :, :]
            xt = xload.tile([P, C], F32, tag="xt")
            nc.sync.dma_start(xt[:], src)
            sq = stats.tile([P, 1], F32, tag="sq")
            sqd = scratch.tile([P, C], F32, tag="sqd")
            nc.scalar.activation(sqd[:], xt[:], AF.Square, accum_out=sq[:])
            nrm = stats.tile([P, 1], F32, tag="nrm")
            nc.scalar.sqrt(nrm[:], sq[:])
            inv = stats.tile([P, 1], F32, tag="inv")
            nc.vector.reciprocal(inv[:], nrm[:])
            xnb = xnp_.tile([P, C], BF16, tag="xnb")
            nc.vector.tensor_scalar_mul(xnb[:], xt[:], inv[:])
            nc.sync.dma_start_transpose(dst, xnb[:])

        # ---- main: m = g - 64*same via augmented matmul; fused reductions ----
        for m in range(NM):
            for g in range(2):
                jbs = list(range(g * 4, g * 4 + 4))
                pts = [psum.tile([P, NB], F32, tag="pt", name="pt") for _ in jbs]
                for c in range(KC + 1):
                    if c < KC:
                        lhsT = xat[:, m, c, :]
                    else:
                        lhsT = ohp[:, m * P:(m + 1) * P]
                    for q, jb in enumerate(jbs):
                        if c < KC:
                            rhs = xnt[:, jb * 4:(jb + 1) * 4, c, :]
                        else:
                            rhs = ohn[:, jb * NB:(jb + 1) * NB]
                        nc.tensor.matmul(pts[q][:], lhsT, rhs,
                                         start=(c == 0), stop=(c == KC))
                for q, jb in enumerate(jbs):
                    col = m * NJ + jb
                    rld = scratch.tile([P, NB], F32, tag="rld")
                    # relu(-m - 63); accum_out = row sum = pos_half partial
                    nc.scalar.activation(rld[:], pts[q][:], AF.Relu,
                                         bias=bneg63[:], scale=-1.0,
                                         accum_out=pos_all[:, col:col + 1])
                    nc.vector.reduce_max(max_all[:, col:col + 1], pts[q][:],
                                         axis=AX.X)

        # ---- tail: per-anchor loss, partition-sum, scale ----
        posg = small.tile([P, NM], F32)
        nc.vector.reduce_sum(posg[:], pos_all.rearrange("p (m j) -> p m j", j=NJ),
                             axis=AX.X)
        maxg = small.tile([P, NM], F32)
        nc.vector.reduce_max(maxg[:], max_all.rearrange("p (m j) -> p m j", j=NJ),
                             axis=AX.X)
        hneg = small.tile([P, NM], F32)
        nc.scalar.activation(hneg[:], maxg[:], AF.Relu, bias=1.0, scale=-1.0)
        diff = small.tile([P, NM], F32)
        nc.vector.tensor_sub(diff[:], posg[:], hneg[:])
        loss = small.tile([P, NM], F32)
        nc.scalar.activation(loss[:], diff[:], AF.Relu, bias=bhalf[:], scale=2.0)
        psc = psum1.tile([1, NM], F32, tag="psc")
        nc.tensor.matmul(psc[:], ones[:], loss[:], start=True, stop=True)
        red = small.tile([1, 1], F32)
        nc.vector.reduce_sum(red[:], psc[:], axis=AX.X)
        outt = small.tile([1, 1], F32)
        nc.scalar.mul(outt[:], red[:], 1.0 / B)
        nc.sync.dma_start(out_d, outt[:])

    nc.compile()
    return nc


_NC = None


def _get_nc():
    global _NC
    if _NC is None:
        _NC = build_kernel()
    return _NC


def make_in_maps(x, label):
    x = np.ascontiguousarray(np.asarray(x, dtype=np.float32))
    label = np.asarray(label).astype(np.int64)
    oh = np.zeros((NLAB, B), dtype=np.float32)
    oh[label, np.arange(B)] = 1.0
    ohp_full = (ALPHA * oh).astype(ml_dtypes.bfloat16)
    ohn_full = (-ALPHA * oh).astype(ml_dtypes.bfloat16)
    in_maps = []
    for c in range(NCORES):
        sl = slice(c * BA, (c + 1) * BA)
        in_maps.append({
            "x": x,
            "xa": np.ascontiguousarray(x[sl]),
            "ohp": np.ascontiguousarray(ohp_full[:, sl]),
            "ohn": ohn_full,
        })
    return in_maps


def kernel(x, label):
    nc = _get_nc()
    res = run_bass_kernel_spmd(nc, make_in_maps(x, label),
                               core_ids=list(range(NCORES)))
    total = sum(float(r["out"][0, 0]) for r in res.results)
    return np.float32(total)


# revision 11
# speedup vs baseline: 1.0820x; 1.0820x over previous
"""Contrastive loss (batch-hard triplet, within batch) on 8 Trainium2 cores.

Math (matches the jax reference):
    xn = x / ||x||_2 (rows)                      [B, C] = [4096, 1024]
    g[i,j] = xn_i . xn_j
    d[i,j] = max(2 - 2 g, 0)   (since ||xn||=1)
    pos_i  = sum_{j: same label, j != i} d[i,j]
    neg_i  = min_{j: diff label} d[i,j]
    loss   = mean(relu(pos_i + 0.5 - neg_i))

Sharding: rows (anchors) split 512/core. Every core builds the full
normalized-transposed xn^T (bf16) and computes its [512, 4096] tile of m.

Label-mask fusion: feature dim is augmented with 64 one-hot columns scaled
+8 on the anchor side and -8 on the rhs side, so the PE produces
    m[i,j] = g[i,j] - 64 * same[i,j]
in the same accumulation group.  Then
    pos_i     = 2 * sum_j relu(-m - 63)          (exact: relu kills diff-label)
    hardest_i = 2 * relu(1 - max_j m)            (max ignores same-label, m<=-63)
    loss_i    = relu(2*(pos_half - hardest_half) + 0.5)
Per-core output is sum(loss_i)/4096; the host adds the 8 partial scalars.
"""

import sys

if "/opt/trn_rl_repo" not in sys.path:
    sys.path.insert(0, "/opt/trn_rl_repo")

from contextlib import ExitStack

import ml_dtypes
import numpy as np

import concourse.bass as bass
import concourse.tile as tile
from concourse import bacc, mybir
from concourse.bass_utils import run_bass_kernel_spmd

B = 4096          # batch rows
C = 1024          # features
NCORES = 8
BA = B // NCORES  # anchors per core = 512
P = 128
KC = C // P       # 8 feature chunks of 128
NT = B // P       # 32 row tiles of full x
NTA = BA // P     # 4 row tiles of anchors
NM = BA // P      # 4 anchor blocks (M=128 each)
NB = 512          # j-block width (moving free dim)
NJ = B // NB      # 8 j blocks
NLAB = 64
ALPHA = 8.0       # onehot scale; ALPHA^2 = 64 = label-offset

F32 = mybir.dt.float32
BF16 = mybir.dt.bfloat16
AF = mybir.ActivationFunctionType
AX = mybir.AxisListType


def build_kernel(loads_gpsimd=True, use_ttr=False):
    # NOTE: use_ttr=True (DVE tensor_tensor_reduce) crashes the device via
    # this toolchain (NRT INTERNAL error) — keep squares on ACT.
    nc = bacc.Bacc("TRN2", target_bir_lowering=False, debug=False,
                   num_devices=NCORES)
    x_d = nc.dram_tensor("x", (B, C), F32, kind="ExternalInput").ap()
    xa_d = nc.dram_tensor("xa", (BA, C), F32, kind="ExternalInput").ap()
    ohp_d = nc.dram_tensor("ohp", (NLAB, BA), BF16, kind="ExternalInput").ap()
    ohn_d = nc.dram_tensor("ohn", (NLAB, B), BF16, kind="ExternalInput").ap()
    out_d = nc.dram_tensor("out", (1, 1), F32, kind="ExternalOutput").ap()

    with tile.TileContext(nc) as tc, ExitStack() as ctx:
        big = ctx.enter_context(tc.tile_pool(name="big", bufs=1))
        xload = ctx.enter_context(tc.tile_pool(name="xload", bufs=4))
        xnp_ = ctx.enter_context(tc.tile_pool(name="xnp", bufs=4))
        stats = ctx.enter_context(tc.tile_pool(name="stats", bufs=8))
        scratch = ctx.enter_context(tc.tile_pool(name="scratch", bufs=2))
        psum = ctx.enter_context(tc.tile_pool(name="psum", bufs=7, space="PSUM"))
        psum1 = ctx.enter_context(tc.tile_pool(name="psum1", bufs=1, space="PSUM"))
        small = ctx.enter_context(tc.tile_pool(name="small", bufs=1))

        # Persistent transposed tensors.
        # xnt[p, t, c, j'] = xn[t*128 + j', c*128 + p]  (bf16)
        xnt = big.tile([P, NT, KC, P], BF16)
        xat = big.tile([P, NTA, KC, P], BF16)
        ohp = big.tile([NLAB, BA], BF16)
        ohn = big.tile([NLAB, B], BF16)
        pos_all = big.tile([P, NM * NJ], F32)
        max_all = big.tile([P, NM * NJ], F32)
        ones = big.tile([P, 1], F32)
        bneg63 = big.tile([P, 1], F32)
        bhalf = big.tile([P, 1], F32)

        nc.sync.dma_start(ohp[:], ohp_d)
        nc.sync.dma_start(ohn[:], ohn_d)
        nc.vector.memset(ones[:], 1.0)
        nc.vector.memset(bneg63[:], -63.0)
        nc.vector.memset(bhalf[:], 0.5)

        # ---- prep: load rows, normalize, bf16, xbar-transpose ----
        # Anchor tiles first: every matmul needs them. Loads go on the
        # gpsimd SWDGE queue so HWDGE transpose waits can't head-of-line
        # block them; transposes alternate between the two HWDGE queues.
        for i in range(NT + NTA):
            t = i - NTA
            if t < 0:
                src = xa_d[(t + NTA) * P:(t + NTA + 1) * P, :]
                dst = xat[:, t + NTA, :, :]
            else:
                src = x_d[t * P:(t + 1) * P, :]
                dst = xnt[:, t, :, :]
            xt = xload.tile([P, C], F32, tag="xt")
            if loads_gpsimd:
                nc.gpsimd.dma_start(xt[:], src)
            else:
                nc.sync.dma_start(xt[:], src)
            sq = stats.tile([P, 1], F32, tag="sq")
            sqd = scratch.tile([P, C], F32, tag="sqd")
            if use_ttr and i % 2 == 1:
                nc.vector.tensor_tensor_reduce(
                    sqd[:], xt[:], xt[:], 1.0, 0.0,
                    mybir.AluOpType.mult, mybir.AluOpType.add, sq[:])
            else:
                nc.scalar.activation(sqd[:], xt[:], AF.Square, accum_out=sq[:])
            nrm = stats.tile([P, 1], F32, tag="nrm")
            nc.scalar.sqrt(nrm[:], sq[:])
            inv = stats.tile([P, 1], F32, tag="inv")
            nc.vector.reciprocal(inv[:], nrm[:])
            xnb = xnp_.tile([P, C], BF16, tag="xnb")
            nc.vector.tensor_scalar_mul(xnb[:], xt[:], inv[:])
            nc.sync.dma_start_transpose(dst, xnb[:])

        # ---- main: m = g - 64*same via augmented matmul; fused reductions ----
        for g in range(2):
            for m in range(NM):
                jbs = list(range(g * 4, g * 4 + 4))
                pts = [psum.tile([P, NB], F32, tag="pt", name="pt") for _ in jbs]
                for c in range(KC + 1):
                    if c < KC:
                        lhsT = xat[:, m, c, :]
                    else:
                        lhsT = ohp[:, m * P:(m + 1) * P]
                    for q, jb in enumerate(jbs):
                        if c < KC:
                            rhs = xnt[:, jb * 4:(jb + 1) * 4, c, :]
                        else:
                            rhs = ohn[:, jb * NB:(jb + 1) * NB]
                        nc.tensor.matmul(pts[q][:], lhsT, rhs,
                                         start=(c == 0), stop=(c == KC))
                for q, jb in enumerate(jbs):
                    col = m * NJ + jb
                    rld = scratch.tile([P, NB], F32, tag="rld")
                    # relu(-m - 63); accum_out = row sum = pos_half partial
                    nc.scalar.activation(rld[:], pts[q][:], AF.Relu,
                                         bias=bneg63[:], scale=-1.0,
                                         accum_out=pos_all[:, col:col + 1])
                    nc.vector.reduce_max(max_all[:, col:col + 1], pts[q][:],
                                         axis=AX.X)

        # ---- tail: per-anchor loss, partition-sum, scale ----
        posg = small.tile([P, NM], F32)
        nc.vector.reduce_sum(posg[:], pos_all.rearrange("p (m j) -> p m j", j=NJ),
                             axis=AX.X)
        maxg = small.tile([P, NM], F32)
        nc.vector.reduce_max(maxg[:], max_all.rearrange("p (m j) -> p m j", j=NJ),
                             axis=AX.X)
        hneg = small.tile([P, NM], F32)
        nc.scalar.activation(hneg[:], maxg[:], AF.Relu, bias=1.0, scale=-1.0)
        diff = small.tile([P, NM], F32)
        nc.vector.tensor_sub(diff[:], posg[:], hneg[:])
        loss = small.tile([P, NM], F32)
        nc.scalar.activation(loss[:], diff[:], AF.Relu, bias=bhalf[:], scale=2.0)
        psc = psum1.tile([1, NM], F32, tag="psc")
        nc.tensor.matmul(psc[:], ones[:], loss[:], start=True, stop=True)
        red = small.tile([1, 1], F32)
        nc.vector.reduce_sum(red[:], psc[:], axis=AX.X)
        outt = small.tile([1, 1], F32)
        nc.scalar.mul(outt[:], red[:], 1.0 / B)
        nc.sync.dma_start(out_d, outt[:])

    nc.compile()
    return nc


_NC = None


def _get_nc():
    global _NC
    if _NC is None:
        _NC = build_kernel()
    return _NC


def make_in_maps(x, label):
    x = np.ascontiguousarray(np.asarray(x, dtype=np.float32))
    label = np.asarray(label).astype(np.int64)
    oh = np.zeros((NLAB, B), dtype=np.float32)
    oh[label, np.arange(B)] = 1.0
    ohp_full = (ALPHA * oh).astype(ml_dtypes.bfloat16)
    ohn_full = (-ALPHA * oh).astype(ml_dtypes.bfloat16)
    in_maps = []
    for c in range(NCORES):
        sl = slice(c * BA, (c + 1) * BA)
        in_maps.append({
            "x": x,
            "xa": np.ascontiguousarray(x[sl]),
            "ohp": np.ascontiguousarray(ohp_full[:, sl]),
            "ohn": ohn_full,
        })
    return in_maps


def kernel(x, label):
    nc = _get_nc()
    res = run_bass_kernel_spmd(nc, make_in_maps(x, label),
                               core_ids=list(range(NCORES)))
    total = sum(float(r["out"][0, 0]) for r in res.results)
    return np.float32(total)


# revision 23
# speedup vs baseline: 1.3277x; 1.2271x over previous
"""Contrastive loss (batch-hard triplet, within batch) on 8 Trainium2 cores.

Math (matches the jax reference):
    xn = x / ||x||_2 (rows)                      [B, C] = [4096, 1024]
    g[i,j] = xn_i . xn_j
    d[i,j] = max(2 - 2 g, 0)   (since ||xn||=1)
    pos_i  = sum_{j: same label, j != i} d[i,j]
    neg_i  = min_{j: diff label} d[i,j]
    loss   = mean(relu(pos_i + 0.5 - neg_i))

Sharding: rows (anchors) split 512/core; every core computes its
[512, 4096] tile of the distance matrix. The host passes x both ways
is unnecessary — only x^T (pure layout transform) plus per-core anchor
slices; ALL math (norms, normalize, matmul, reductions) runs on device.

Device pipeline, per j-slice s of 512 columns (1 anchor slice + 8 full):
  load xT chunks [128, 512] f32 -> square (ACT/DVE) -> PE ones-matmul
  column-reduce -> sq [1,512] -> sqrt -> 1/x -> PE broadcast to [128,512]
  -> DVE multiply (bf16 out) -> xnt_s [128, 8, 512]  (k-major, no xbar!)

Label-mask fusion: 64 one-hot rows scaled +8 (anchor side) and -8 (rhs)
are appended to the contraction, so the PE produces
    m[i,j] = g[i,j] - 64 * same[i,j]
in one accumulation group. Then per matmul output tile:
    pos_half = sum_j relu(-m - 63)     (one ACT op, accum_out)
    mx       = max_j m                 (one DVE reduce)
    loss_i   = relu(2*(pos_half - relu(1 - mx)) + 0.5)
Per-core output is sum(loss_i)/4096; the host adds the 8 partials.
"""

import sys

if "/opt/trn_rl_repo" not in sys.path:
    sys.path.insert(0, "/opt/trn_rl_repo")

from contextlib import ExitStack

import ml_dtypes
import numpy as np

import concourse.bass as bass
import concourse.tile as tile
from concourse import bacc, mybir
from concourse.bass_utils import run_bass_kernel_spmd

B = 4096          # batch rows
C = 1024          # features
NCORES = 8
BA = B // NCORES  # anchors per core = 512
P = 128
KC = C // P       # 8 feature chunks of 128
NB = 512          # j-slice width
NJ = B // NB      # 8 j slices
NM = BA // P      # 4 anchor blocks (M=128 each)
NLAB = 64

F32 = mybir.dt.float32
BF16 = mybir.dt.bfloat16
FP8 = mybir.dt.float8e4
AF = mybir.ActivationFunctionType
AX = mybir.AxisListType

# fp8 path: matmul operands are (16*xn) in fp8e4 with DoubleRow pairs, so the
# PSUM holds 256*(g - 64*same); one-hots are +-128; post-ops rescale by 1/256.
import os
USE_FP8 = os.environ.get("CONTRASTIVE_FP8", "0") == "1"
ALPHA = 128.0 if USE_FP8 else 8.0   # onehot scale; product = -64 * XSCALE^2
XSCALE = 16.0 if USE_FP8 else 1.0
XDT = FP8 if USE_FP8 else BF16
PSC = 1.0 / (XSCALE * XSCALE)       # PSUM -> m rescale


def build_kernel():
    nc = bacc.Bacc("TRN2", target_bir_lowering=False, debug=False,
                   num_devices=NCORES)
    xt_d = nc.dram_tensor("xT", (C, B), F32, kind="ExternalInput").ap()
    xat_d = nc.dram_tensor("xaT", (C, BA), F32, kind="ExternalInput").ap()
    ohp_d = nc.dram_tensor("ohp", (NLAB, BA), XDT, kind="ExternalInput").ap()
    ohn_d = nc.dram_tensor("ohn", (NLAB, B), XDT, kind="ExternalInput").ap()
    out_d = nc.dram_tensor("out", (1, 1), F32, kind="ExternalOutput").ap()

    with tile.TileContext(nc) as tc, ExitStack() as ctx:
        big = ctx.enter_context(tc.tile_pool(name="big", bufs=1))
        xload = ctx.enter_context(tc.tile_pool(name="xload", bufs=16))
        sqp = ctx.enter_context(tc.tile_pool(name="sqp", bufs=3))
        stats = ctx.enter_context(tc.tile_pool(name="stats", bufs=3))
        scratch = ctx.enter_context(tc.tile_pool(name="scratch", bufs=2))
        psum = ctx.enter_context(tc.tile_pool(name="psum", bufs=5, space="PSUM"))
        psum1 = ctx.enter_context(tc.tile_pool(name="psum1", bufs=1, space="PSUM"))
        small = ctx.enter_context(tc.tile_pool(name="small", bufs=1))

        # xnt_s[p, c, j] = XSCALE * xn[s*512 + j, c*128 + p], one tile per slice
        xnts = [big.tile([P, KC, NB], XDT, name=f"xnt{s}", tag=f"xnt{s}")
                for s in range(NJ)]
        xat = big.tile([P, KC, BA], XDT)
        ohp = big.tile([NLAB, BA], XDT)
        ohn = big.tile([NLAB, B], XDT)
        pos_all = big.tile([P, NM * NJ], F32)
        max_all = big.tile([P, NM * NJ], F32)
        ones = big.tile([P, 1], F32)
        ones1 = big.tile([1, P], F32)
        bneg63 = big.tile([P, 1], F32)
        bhalf = big.tile([P, 1], F32)

        nc.sync.dma_start(ohp[:], ohp_d)
        nc.sync.dma_start(ohn[:], ohn_d)
        nc.vector.memset(ones[:], 1.0)
        nc.vector.memset(ones1[:], 1.0)
        nc.vector.memset(bneg63[:], -63.0)
        nc.vector.memset(bhalf[:], 0.5)

        def prep_slice(s):
            """s = -1: anchor slice -> xat; else j-slice s -> xnts[s]."""
            if s < 0:
                srcs = [xat_d[c * P:(c + 1) * P, :] for c in range(KC)]
                dst = xat
                w = BA
            else:
                srcs = [xt_d[c * P:(c + 1) * P, s * NB:(s + 1) * NB]
                        for c in range(KC)]
                dst = xnts[s]
                w = NB
            lts = []
            sq_ps = psum1.tile([1, NB], F32, tag="sqps", name="sq_ps")
            for c in range(KC):
                lt = xload.tile([P, NB], F32, tag="lt", name="lt")
                nc.sync.dma_start(lt[:, :w], srcs[c])
                xsq = sqp.tile([P, NB], F32, tag="xsq", name="xsq")
                nc.scalar.square(xsq[:, :w], lt[:, :w])
                nc.tensor.matmul(sq_ps[:, :w], ones[:], xsq[:, :w],
                                 start=(c == 0), stop=(c == KC - 1))
                lts.append(lt)
            nrm = stats.tile([1, NB], F32, tag="nrm", name="nrm")
            # nrm = sqrt(sq)/XSCALE, so inv = XSCALE/||x|| folds the fp8 scale
            nc.scalar.activation(nrm[:, :w], sq_ps[:, :w], AF.Sqrt,
                                 scale=PSC)
            inv = stats.tile([1, NB], F32, tag="inv", name="inv")
            nc.vector.reciprocal(inv[:, :w], nrm[:, :w])
            bc_ps = psum1.tile([P, NB], F32, tag="bcps", name="bc_ps")
            nc.tensor.matmul(bc_ps[:, :w], ones1[:], inv[:, :w],
                             start=True, stop=True)
            invb = scratch.tile([P, NB], F32, tag="invb", name="invb")
            nc.vector.tensor_copy(invb[:, :w], bc_ps[:, :w])
            for c in range(KC):
                nc.vector.tensor_mul(dst[:, c, :w], lts[c][:, :w], invb[:, :w])

        prep_slice(-1)

        # ---- main: m = g - 64*same via augmented matmul; fused reductions ----
        for jb in range(NJ):
            prep_slice(jb)
            pts = [psum.tile([P, NB], F32, tag="pt", name="pt")
                   for _ in range(NM)]
            if USE_FP8:
                for cp in range(KC // 2 + 1):
                    for m in range(NM):
                        if cp < KC // 2:
                            lhsT = xat[:, 2 * cp:2 * cp + 2, m * P:(m + 1) * P]
                            rhs = xnts[jb][:, 2 * cp:2 * cp + 2, :]
                            pm = mybir.MatmulPerfMode.DoubleRow
                        else:
                            lhsT = ohp[:, m * P:(m + 1) * P]
                            rhs = ohn[:, jb * NB:(jb + 1) * NB]
                            pm = None
                        nc.tensor.matmul(pts[m][:], lhsT, rhs, perf_mode=pm,
                                         start=(cp == 0), stop=(cp == KC // 2))
            else:
                for c in range(KC + 1):
                    for m in range(NM):
                        if c < KC:
                            lhsT = xat[:, c, m * P:(m + 1) * P]
                            rhs = xnts[jb][:, c, :]
                        else:
                            lhsT = ohp[:, m * P:(m + 1) * P]
                            rhs = ohn[:, jb * NB:(jb + 1) * NB]
                        nc.tensor.matmul(pts[m][:], lhsT, rhs,
                                         start=(c == 0), stop=(c == KC))
            for m in range(NM):
                col = m * NJ + jb
                rld = scratch.tile([P, NB], F32, tag="rld", name="rld")
                nc.scalar.activation(rld[:], pts[m][:], AF.Relu,
                                     bias=bneg63[:], scale=-PSC,
                                     accum_out=pos_all[:, col:col + 1])
                nc.vector.reduce_max(max_all[:, col:col + 1], pts[m][:],
                                     axis=AX.X)

        # ---- tail: per-anchor loss, partition-sum, scale ----
        posg = small.tile([P, NM], F32)
        nc.vector.reduce_sum(posg[:], pos_all.rearrange("p (m j) -> p m j", j=NJ),
                             axis=AX.X)
        maxg = small.tile([P, NM], F32)
        nc.vector.reduce_max(maxg[:], max_all.rearrange("p (m j) -> p m j", j=NJ),
                             axis=AX.X)
        hneg = small.tile([P, NM], F32)
        nc.scalar.activation(hneg[:], maxg[:], AF.Relu, bias=1.0, scale=-PSC)
        diff = small.tile([P, NM], F32)
        nc.vector.tensor_sub(diff[:], posg[:], hneg[:])
        loss = small.tile([P, NM], F32)
        nc.scalar.activation(loss[:], diff[:], AF.Relu, bias=bhalf[:], scale=2.0)
        psc = psum1.tile([1, NM], F32, tag="psc")
        nc.tensor.matmul(psc[:], ones[:], loss[:], start=True, stop=True)
        red = small.tile([1, 1], F32)
        nc.vector.reduce_sum(red[:], psc[:], axis=AX.X)
        outt = small.tile([1, 1], F32)
        nc.scalar.mul(outt[:], red[:], 1.0 / B)
        nc.sync.dma_start(out_d, outt[:])

    nc.compile()
    return nc


_NC = None


def _get_nc():
    global _NC
    if _NC is None:
        _NC = build_kernel()
    return _NC


def make_in_maps(x, label):
    x = np.ascontiguousarray(np.asarray(x, dtype=np.float32))
    label = np.asarray(label).astype(np.int64)
    xT = np.ascontiguousarray(x.T)
    np_xdt = ml_dtypes.float8_e4m3 if USE_FP8 else ml_dtypes.bfloat16
    oh = np.zeros((NLAB, B), dtype=np.float32)
    oh[label, np.arange(B)] = 1.0
    ohp_full = (ALPHA * oh).astype(np_xdt)
    ohn_full = (-ALPHA * oh).astype(np_xdt)
    in_maps = []
    for c in range(NCORES):
        sl = slice(c * BA, (c + 1) * BA)
        in_maps.append({
            "xT": xT,
            "xaT": np.ascontiguousarray(xT[:, sl]),
            "ohp": np.ascontiguousarray(ohp_full[:, sl]),
            "ohn": ohn_full,
        })
    return in_maps


def kernel(x, label):
    nc = _get_nc()
    res = run_bass_kernel_spmd(nc, make_in_maps(x, label),
                               core_ids=list(range(NCORES)))
    total = sum(float(r["out"][0, 0]) for r in res.results)
    return np.float32(total)


# revision 27
# speedup vs baseline: 1.4363x; 1.0818x over previous
"""Contrastive loss (batch-hard triplet, within batch) on 8 Trainium2 cores.

Math (matches the jax reference):
    xn = x / ||x||_2 (rows)                      [B, C] = [4096, 1024]
    g[i,j] = xn_i . xn_j
    d[i,j] = max(2 - 2 g, 0)   (since ||xn||=1)
    pos_i  = sum_{j: same label, j != i} d[i,j]
    neg_i  = min_{j: diff label} d[i,j]
    loss   = mean(relu(pos_i + 0.5 - neg_i))

Sharding: rows (anchors) split 512/core; every core computes its
[512, 4096] tile of the distance matrix. The host passes x both ways
is unnecessary — only x^T (pure layout transform) plus per-core anchor
slices; ALL math (norms, normalize, matmul, reductions) runs on device.

Device pipeline, per j-slice s of 512 columns (1 anchor slice + 8 full):
  load xT chunks [128, 512] f32 -> square (ACT/DVE) -> PE ones-matmul
  column-reduce -> sq [1,512] -> sqrt -> 1/x -> PE broadcast to [128,512]
  -> DVE multiply (bf16 out) -> xnt_s [128, 8, 512]  (k-major, no xbar!)

Label-mask fusion: 64 one-hot rows scaled +8 (anchor side) and -8 (rhs)
are appended to the contraction, so the PE produces
    m[i,j] = g[i,j] - 64 * same[i,j]
in one accumulation group. Then per matmul output tile:
    pos_half = sum_j relu(-m - 63)     (one ACT op, accum_out)
    mx       = max_j m                 (one DVE reduce)
    loss_i   = relu(2*(pos_half - relu(1 - mx)) + 0.5)
Per-core output is sum(loss_i)/4096; the host adds the 8 partials.
"""

import sys

if "/opt/trn_rl_repo" not in sys.path:
    sys.path.insert(0, "/opt/trn_rl_repo")

from contextlib import ExitStack

import ml_dtypes
import numpy as np

import concourse.bass as bass
import concourse.tile as tile
from concourse import bacc, mybir
from concourse.bass_utils import run_bass_kernel_spmd

B = 4096          # batch rows
C = 1024          # features
NCORES = 8
BA = B // NCORES  # anchors per core = 512
P = 128
KC = C // P       # 8 feature chunks of 128
NB = 512          # j-slice width
NJ = B // NB      # 8 j slices
NM = BA // P      # 4 anchor blocks (M=128 each)
NLAB = 64

F32 = mybir.dt.float32
BF16 = mybir.dt.bfloat16
FP8 = mybir.dt.float8e4
AF = mybir.ActivationFunctionType
AX = mybir.AxisListType

# fp8 path: matmul operands are (16*xn) in fp8e4 with DoubleRow pairs, so the
# PSUM holds 256*(g - 64*same); one-hots are +-128; post-ops rescale by 1/256.
import os
USE_FP8 = os.environ.get("CONTRASTIVE_FP8", "1") == "1"
ALPHA = 128.0 if USE_FP8 else 8.0   # onehot scale; product = -64 * XSCALE^2
XSCALE = 16.0 if USE_FP8 else 1.0
XDT = FP8 if USE_FP8 else BF16
PSC = 1.0 / (XSCALE * XSCALE)       # PSUM -> m rescale


def build_kernel():
    nc = bacc.Bacc("TRN2", target_bir_lowering=False, debug=False,
                   num_devices=NCORES)
    xt_d = nc.dram_tensor("xT", (C, B), F32, kind="ExternalInput").ap()
    xat_d = nc.dram_tensor("xaT", (C, BA), F32, kind="ExternalInput").ap()
    ohp_d = nc.dram_tensor("ohp", (NLAB, BA), XDT, kind="ExternalInput").ap()
    ohn_d = nc.dram_tensor("ohn", (NLAB, B), XDT, kind="ExternalInput").ap()
    out_d = nc.dram_tensor("out", (1, 1), F32, kind="ExternalOutput").ap()

    with tile.TileContext(nc) as tc, ExitStack() as ctx:
        big = ctx.enter_context(tc.tile_pool(name="big", bufs=1))
        xload = ctx.enter_context(tc.tile_pool(name="xload", bufs=16))
        sqp = ctx.enter_context(tc.tile_pool(name="sqp", bufs=3))
        stats = ctx.enter_context(tc.tile_pool(name="stats", bufs=3))
        scratch = ctx.enter_context(tc.tile_pool(name="scratch", bufs=2))
        psum = ctx.enter_context(tc.tile_pool(name="psum", bufs=5, space="PSUM"))
        psum1 = ctx.enter_context(tc.tile_pool(name="psum1", bufs=1, space="PSUM"))
        small = ctx.enter_context(tc.tile_pool(name="small", bufs=1))

        # xnt_s[p, c, j] = XSCALE * xn[s*512 + j, c*128 + p], one tile per slice
        xnts = [big.tile([P, KC, NB], XDT, name=f"xnt{s}", tag=f"xnt{s}")
                for s in range(NJ)]
        xat = big.tile([P, KC, BA], XDT)
        ohp = big.tile([NLAB, BA], XDT)
        ohn = big.tile([NLAB, B], XDT)
        pos_all = big.tile([P, NM * NJ], F32)
        max_all = big.tile([P, NM * NJ], F32)
        ones = big.tile([P, 1], F32)
        ones1 = big.tile([1, P], F32)
        ones128 = big.tile([P, P], BF16)
        bneg63 = big.tile([P, 1], F32)
        bhalf = big.tile([P, 1], F32)

        nc.sync.dma_start(ohp[:], ohp_d)
        nc.sync.dma_start(ohn[:], ohn_d)
        nc.vector.memset(ones[:], 1.0)
        nc.vector.memset(ones1[:], 1.0)
        nc.vector.memset(ones128[:], 1.0)
        nc.vector.memset(bneg63[:], -63.0)
        nc.vector.memset(bhalf[:], 0.5)

        def prep_slice(s):
            """s = -1: anchor slice -> xat; else j-slice s -> xnts[s]."""
            if s < 0:
                srcs = [xat_d[c * P:(c + 1) * P, :] for c in range(KC)]
                dst = xat
                w = BA
            else:
                srcs = [xt_d[c * P:(c + 1) * P, s * NB:(s + 1) * NB]
                        for c in range(KC)]
                dst = xnts[s]
                w = NB
            lts = []
            sq_ps = psum1.tile([P, NB], F32, tag="sqps", name="sq_ps")
            for c in range(KC):
                lt = xload.tile([P, NB], F32, tag="lt", name="lt")
                nc.sync.dma_start(lt[:, :w], srcs[c])
                xsq = sqp.tile([P, NB], BF16, tag="xsq", name="xsq")
                if c < 4:
                    nc.scalar.square(xsq[:, :w], lt[:, :w])
                elif c < 7:
                    nc.gpsimd.tensor_mul(xsq[:, :w], lt[:, :w], lt[:, :w])
                else:
                    nc.vector.tensor_mul(xsq[:, :w], lt[:, :w], lt[:, :w])
                # M=128 ones weights: every out row = column-sums; row 0 used
                nc.tensor.matmul(sq_ps[:, :w], ones128[:], xsq[:, :w],
                                 start=(c == 0), stop=(c == KC - 1))
                lts.append(lt)
            nrm = stats.tile([1, NB], F32, tag="nrm", name="nrm")
            # nrm = sqrt(sq)/XSCALE, so inv = XSCALE/||x|| folds the fp8 scale
            nc.scalar.activation(nrm[:, :w], sq_ps[0:1, :w], AF.Sqrt,
                                 scale=PSC)
            bc_ps = psum1.tile([P, NB], F32, tag="bcps", name="bc_ps")
            nc.tensor.matmul(bc_ps[:, :w], ones1[:], nrm[:, :w],
                             start=True, stop=True)
            invb = scratch.tile([P, NB], F32, tag="invb", name="invb")
            nc.vector.reciprocal(invb[:, :w], bc_ps[:, :w])
            for c in range(KC):
                eng = nc.vector if c < 4 else nc.gpsimd
                eng.tensor_mul(dst[:, c, :w], lts[c][:, :w], invb[:, :w])

        prep_slice(-1)

        # ---- main: m = g - 64*same via augmented matmul; fused reductions ----
        for jb in range(NJ):
            prep_slice(jb)
            pts = [psum.tile([P, NB], F32, tag="pt", name="pt")
                   for _ in range(NM)]
            if USE_FP8:
                for cp in range(KC // 2 + 1):
                    for m in range(NM):
                        if cp < KC // 2:
                            lhsT = xat[:, 2 * cp:2 * cp + 2, m * P:(m + 1) * P]
                            rhs = xnts[jb][:, 2 * cp:2 * cp + 2, :]
                            pm = mybir.MatmulPerfMode.DoubleRow
                        else:
                            lhsT = ohp[:, m * P:(m + 1) * P]
                            rhs = ohn[:, jb * NB:(jb + 1) * NB]
                            pm = None
                        nc.tensor.matmul(pts[m][:], lhsT, rhs, perf_mode=pm,
                                         start=(cp == 0), stop=(cp == KC // 2))
            else:
                for c in range(KC + 1):
                    for m in range(NM):
                        if c < KC:
                            lhsT = xat[:, c, m * P:(m + 1) * P]
                            rhs = xnts[jb][:, c, :]
                        else:
                            lhsT = ohp[:, m * P:(m + 1) * P]
                            rhs = ohn[:, jb * NB:(jb + 1) * NB]
                        nc.tensor.matmul(pts[m][:], lhsT, rhs,
                                         start=(c == 0), stop=(c == KC))
            for m in range(NM):
                col = m * NJ + jb
                rld = scratch.tile([P, NB], F32, tag="rld", name="rld")
                nc.scalar.activation(rld[:], pts[m][:], AF.Relu,
                                     bias=bneg63[:], scale=-PSC,
                                     accum_out=pos_all[:, col:col + 1])
                nc.vector.reduce_max(max_all[:, col:col + 1], pts[m][:],
                                     axis=AX.X)

        # ---- tail: per-anchor loss, partition-sum, scale ----
        posg = small.tile([P, NM], F32)
        nc.vector.reduce_sum(posg[:], pos_all.rearrange("p (m j) -> p m j", j=NJ),
                             axis=AX.X)
        maxg = small.tile([P, NM], F32)
        nc.vector.reduce_max(maxg[:], max_all.rearrange("p (m j) -> p m j", j=NJ),
                             axis=AX.X)
        hneg = small.tile([P, NM], F32)
        nc.scalar.activation(hneg[:], maxg[:], AF.Relu, bias=1.0, scale=-PSC)
        diff = small.tile([P, NM], F32)
        nc.vector.tensor_sub(diff[:], posg[:], hneg[:])
        loss = small.tile([P, NM], F32)
        nc.scalar.activation(loss[:], diff[:], AF.Relu, bias=bhalf[:], scale=2.0)
        psc = psum1.tile([1, NM], F32, tag="psc")
        nc.tensor.matmul(psc[:], ones[:], loss[:], start=True, stop=True)
        red = small.tile([1, 1], F32)
        nc.vector.reduce_sum(red[:], psc[:], axis=AX.X)
        outt = small.tile([1, 1], F32)
        nc.scalar.mul(outt[:], red[:], 1.0 / B)
        nc.sync.dma_start(out_d, outt[:])

    nc.compile()
    return nc


_NC = None


def _get_nc():
    global _NC
    if _NC is None:
        _NC = build_kernel()
    return _NC


def make_in_maps(x, label):
    x = np.ascontiguousarray(np.asarray(x, dtype=np.float32))
    label = np.asarray(label).astype(np.int64)
    xT = np.ascontiguousarray(x.T)
    np_xdt = ml_dtypes.float8_e4m3 if USE_FP8 else ml_dtypes.bfloat16
    oh = np.zeros((NLAB, B), dtype=np.float32)
    oh[label, np.arange(B)] = 1.0
    ohp_full = (ALPHA * oh).astype(np_xdt)
    ohn_full = (-ALPHA * oh).astype(np_xdt)
    in_maps = []
    for c in range(NCORES):
        sl = slice(c * BA, (c + 1) * BA)
        in_maps.append({
            "xT": xT,
            "xaT": np.ascontiguousarray(xT[:, sl]),
            "ohp": np.ascontiguousarray(ohp_full[:, sl]),
            "ohn": ohn_full,
        })
    return in_maps


def kernel(x, label):
    nc = _get_nc()
    res = run_bass_kernel_spmd(nc, make_in_maps(x, label),
                               core_ids=list(range(NCORES)))
    total = sum(float(r["out"][0, 0]) for r in res.results)
    return np.float32(total)


# revision 30
# speedup vs baseline: 1.5011x; 1.0451x over previous
"""Contrastive loss (batch-hard triplet, within batch) on 8 Trainium2 cores.

Math (matches the jax reference):
    xn = x / ||x||_2 (rows)                      [B, C] = [4096, 1024]
    g[i,j] = xn_i . xn_j
    d[i,j] = max(2 - 2 g, 0)   (since ||xn||=1)
    pos_i  = sum_{j: same label, j != i} d[i,j]
    neg_i  = min_{j: diff label} d[i,j]
    loss   = mean(relu(pos_i + 0.5 - neg_i))

Sharding: rows (anchors) split 512/core; every core computes its
[512, 4096] tile of the distance matrix. The host passes x both ways
is unnecessary — only x^T (pure layout transform) plus per-core anchor
slices; ALL math (norms, normalize, matmul, reductions) runs on device.

Device pipeline, per j-slice s of 512 columns (1 anchor slice + 8 full):
  load xT chunks [128, 512] f32 -> square (ACT/DVE) -> PE ones-matmul
  column-reduce -> sq [1,512] -> sqrt -> 1/x -> PE broadcast to [128,512]
  -> DVE multiply (bf16 out) -> xnt_s [128, 8, 512]  (k-major, no xbar!)

Label-mask fusion: 64 one-hot rows scaled +8 (anchor side) and -8 (rhs)
are appended to the contraction, so the PE produces
    m[i,j] = g[i,j] - 64 * same[i,j]
in one accumulation group. Then per matmul output tile:
    pos_half = sum_j relu(-m - 63)     (one ACT op, accum_out)
    mx       = max_j m                 (one DVE reduce)
    loss_i   = relu(2*(pos_half - relu(1 - mx)) + 0.5)
Per-core output is sum(loss_i)/4096; the host adds the 8 partials.
"""

import sys

if "/opt/trn_rl_repo" not in sys.path:
    sys.path.insert(0, "/opt/trn_rl_repo")

from contextlib import ExitStack

import ml_dtypes
import numpy as np

import concourse.bass as bass
import concourse.tile as tile
from concourse import bacc, mybir
from concourse.bass_utils import run_bass_kernel_spmd

B = 4096          # batch rows
C = 1024          # features
NCORES = 8
BA = B // NCORES  # anchors per core = 512
P = 128
KC = C // P       # 8 feature chunks of 128
NB = 512          # j-slice width
NJ = B // NB      # 8 j slices
NM = BA // P      # 4 anchor blocks (M=128 each)
NLAB = 64

F32 = mybir.dt.float32
BF16 = mybir.dt.bfloat16
FP8 = mybir.dt.float8e4
AF = mybir.ActivationFunctionType
AX = mybir.AxisListType

# fp8 path: matmul operands are (16*xn) in fp8e4 with DoubleRow pairs, so the
# PSUM holds 256*(g - 64*same); one-hots are +-128; post-ops rescale by 1/256.
import os
USE_FP8 = os.environ.get("CONTRASTIVE_FP8", "0") == "1"
ALPHA = 128.0 if USE_FP8 else 8.0   # onehot scale; product = -64 * XSCALE^2
XSCALE = 16.0 if USE_FP8 else 1.0
XDT = FP8 if USE_FP8 else BF16
PSC = 1.0 / (XSCALE * XSCALE)       # PSUM -> m rescale


def build_kernel():
    nc = bacc.Bacc("TRN2", target_bir_lowering=False, debug=False,
                   num_devices=NCORES)
    xt_d = nc.dram_tensor("xT", (C, B), F32, kind="ExternalInput").ap()
    xat_d = nc.dram_tensor("xaT", (C, BA), F32, kind="ExternalInput").ap()
    ohp_d = nc.dram_tensor("ohp", (NLAB, BA), XDT, kind="ExternalInput").ap()
    ohn_d = nc.dram_tensor("ohn", (NLAB, B), XDT, kind="ExternalInput").ap()
    out_d = nc.dram_tensor("out", (1, 1), F32, kind="ExternalOutput").ap()

    with tile.TileContext(nc) as tc, ExitStack() as ctx:
        big = ctx.enter_context(tc.tile_pool(name="big", bufs=1))
        xload = ctx.enter_context(tc.tile_pool(name="xload", bufs=16))
        sqp = ctx.enter_context(tc.tile_pool(name="sqp", bufs=3))
        stats = ctx.enter_context(tc.tile_pool(name="stats", bufs=3))
        scratch = ctx.enter_context(tc.tile_pool(name="scratch", bufs=2))
        psum = ctx.enter_context(tc.tile_pool(name="psum", bufs=5, space="PSUM"))
        psum1 = ctx.enter_context(tc.tile_pool(name="psum1", bufs=1, space="PSUM"))
        small = ctx.enter_context(tc.tile_pool(name="small", bufs=1))

        # xnt_s[p, c, j] = XSCALE * xn[s*512 + j, c*128 + p], one tile per slice
        xnts = [big.tile([P, KC, NB], XDT, name=f"xnt{s}", tag=f"xnt{s}")
                for s in range(NJ)]
        xat = big.tile([P, KC, BA], XDT)
        ohp = big.tile([NLAB, BA], XDT)
        ohn = big.tile([NLAB, B], XDT)
        pos_all = big.tile([P, NM * NJ], F32)
        max_all = big.tile([P, NM * NJ], F32)
        ones = big.tile([P, 1], F32)
        ones1 = big.tile([1, P], F32)
        ones128 = big.tile([P, P], BF16)
        bneg63 = big.tile([P, 1], F32)
        bhalf = big.tile([P, 1], F32)

        nc.sync.dma_start(ohp[:], ohp_d)
        nc.sync.dma_start(ohn[:], ohn_d)
        nc.vector.memset(ones[:], 1.0)
        nc.vector.memset(ones1[:], 1.0)
        nc.vector.memset(ones128[:], 1.0)
        nc.vector.memset(bneg63[:], -63.0)
        nc.vector.memset(bhalf[:], 0.5)

        def prep_slice(s):
            """s = -1: anchor slice -> xat; else j-slice s -> xnts[s]."""
            if s < 0:
                srcs = [xat_d[c * P:(c + 1) * P, :] for c in range(KC)]
                dst = xat
                w = BA
            else:
                srcs = [xt_d[c * P:(c + 1) * P, s * NB:(s + 1) * NB]
                        for c in range(KC)]
                dst = xnts[s]
                w = NB
            lts = []
            sq_ps = psum1.tile([P, NB], F32, tag="sqps", name="sq_ps")
            for c in range(KC):
                lt = xload.tile([P, NB], F32, tag="lt", name="lt")
                nc.sync.dma_start(lt[:, :w], srcs[c])
                xsq = sqp.tile([P, NB], BF16, tag="xsq", name="xsq")
                if c < 5:
                    nc.scalar.square(xsq[:, :w], lt[:, :w])
                elif c < 7:
                    nc.gpsimd.tensor_mul(xsq[:, :w], lt[:, :w], lt[:, :w])
                else:
                    nc.vector.tensor_mul(xsq[:, :w], lt[:, :w], lt[:, :w])
                # M=128 ones weights: every out row = column-sums; row 0 used
                nc.tensor.matmul(sq_ps[:, :w], ones128[:], xsq[:, :w],
                                 start=(c == 0), stop=(c == KC - 1))
                lts.append(lt)
            nrm = stats.tile([1, NB], F32, tag="nrm", name="nrm")
            # nrm = sqrt(sq)/XSCALE, so inv = XSCALE/||x|| folds the fp8 scale
            nc.scalar.activation(nrm[:, :w], sq_ps[0:1, :w], AF.Sqrt,
                                 scale=PSC)
            bc_ps = psum1.tile([P, NB], F32, tag="bcps", name="bc_ps")
            nc.tensor.matmul(bc_ps[:, :w], ones1[:], nrm[:, :w],
                             start=True, stop=True)
            invb = scratch.tile([P, NB], F32, tag="invb", name="invb")
            nc.vector.reciprocal(invb[:, :w], bc_ps[:, :w])
            for c in range(KC):
                eng = nc.vector if c < 5 else nc.gpsimd
                eng.tensor_mul(dst[:, c, :w], lts[c][:, :w], invb[:, :w])

        prep_slice(-1)

        # ---- main: m = g - 64*same via augmented matmul; fused reductions ----
        for jb in range(NJ):
            prep_slice(jb)
            pts = [psum.tile([P, NB], F32, tag="pt", name="pt")
                   for _ in range(NM)]
            if USE_FP8:
                for cp in range(KC // 2 + 1):
                    for m in range(NM):
                        if cp < KC // 2:
                            lhsT = xat[:, 2 * cp:2 * cp + 2, m * P:(m + 1) * P]
                            rhs = xnts[jb][:, 2 * cp:2 * cp + 2, :]
                            pm = mybir.MatmulPerfMode.DoubleRow
                        else:
                            lhsT = ohp[:, m * P:(m + 1) * P]
                            rhs = ohn[:, jb * NB:(jb + 1) * NB]
                            pm = None
                        nc.tensor.matmul(pts[m][:], lhsT, rhs, perf_mode=pm,
                                         start=(cp == 0), stop=(cp == KC // 2))
            else:
                for c in range(KC + 1):
                    for m in range(NM):
                        if c < KC:
                            lhsT = xat[:, c, m * P:(m + 1) * P]
                            rhs = xnts[jb][:, c, :]
                        else:
                            lhsT = ohp[:, m * P:(m + 1) * P]
                            rhs = ohn[:, jb * NB:(jb + 1) * NB]
                        nc.tensor.matmul(pts[m][:], lhsT, rhs,
                                         start=(c == 0), stop=(c == KC))
            for m in range(NM):
                col = m * NJ + jb
                rld = scratch.tile([P, NB], F32, tag="rld", name="rld")
                nc.scalar.activation(rld[:], pts[m][:], AF.Relu,
                                     bias=bneg63[:], scale=-PSC,
                                     accum_out=pos_all[:, col:col + 1])
                nc.vector.reduce_max(max_all[:, col:col + 1], pts[m][:],
                                     axis=AX.X)

        # ---- tail: per-anchor loss, partition-sum, scale ----
        posg = small.tile([P, NM], F32)
        nc.vector.reduce_sum(posg[:], pos_all.rearrange("p (m j) -> p m j", j=NJ),
                             axis=AX.X)
        maxg = small.tile([P, NM], F32)
        nc.vector.reduce_max(maxg[:], max_all.rearrange("p (m j) -> p m j", j=NJ),
                             axis=AX.X)
        hneg = small.tile([P, NM], F32)
        nc.scalar.activation(hneg[:], maxg[:], AF.Relu, bias=1.0, scale=-PSC)
        diff = small.tile([P, NM], F32)
        nc.vector.tensor_sub(diff[:], posg[:], hneg[:])
        loss = small.tile([P, NM], F32)
        nc.scalar.activation(loss[:], diff[:], AF.Relu, bias=bhalf[:], scale=2.0)
        psc = psum1.tile([1, NM], F32, tag="psc")
        nc.tensor.matmul(psc[:], ones[:], loss[:], start=True, stop=True)
        red = small.tile([1, 1], F32)
        nc.vector.reduce_sum(red[:], psc[:], axis=AX.X)
        outt = small.tile([1, 1], F32)
        nc.scalar.mul(outt[:], red[:], 1.0 / B)
        nc.sync.dma_start(out_d, outt[:])

    nc.compile()
    return nc


_NC = None


def _get_nc():
    global _NC
    if _NC is None:
        _NC = build_kernel()
    return _NC


def make_in_maps(x, label):
    x = np.ascontiguousarray(np.asarray(x, dtype=np.float32))
    label = np.asarray(label).astype(np.int64)
    xT = np.ascontiguousarray(x.T)
    np_xdt = ml_dtypes.float8_e4m3 if USE_FP8 else ml_dtypes.bfloat16
    oh = np.zeros((NLAB, B), dtype=np.float32)
    oh[label, np.arange(B)] = 1.0
    ohp_full = (ALPHA * oh).astype(np_xdt)
    ohn_full = (-ALPHA * oh).astype(np_xdt)
    in_maps = []
    for c in range(NCORES):
        sl = slice(c * BA, (c + 1) * BA)
        in_maps.append({
            "xT": xT,
            "xaT": np.ascontiguousarray(xT[:, sl]),
            "ohp": np.ascontiguousarray(ohp_full[:, sl]),
            "ohn": ohn_full,
        })
    return in_maps


def kernel(x, label):
    nc = _get_nc()
    res = run_bass_kernel_spmd(nc, make_in_maps(x, label),
                               core_ids=list(range(NCORES)))
    total = sum(float(r["out"][0, 0]) for r in res.results)
    return np.float32(total)
